# revision 4
# baseline (speedup 1.0000x reference)
"""Grouped 3x3 SAME conv on 8 Trainium2 NeuronCores.

Problem: x[16,56,56,256] NHWC, 8 groups of 32->64 channels, 3x3 SAME,
out[16,56,56,512], fp32.

Strategy (hardcoded):
  - Data-parallel over batch: core i handles images [2i, 2i+1].
  - fp16 operands (fp32 PSUM accumulate, rel err ~3e-4). Input DRAM rows
    are zero-bordered padded channel rows [58 zeros | img 3364 | 58 zeros]
    so the three kh-shifted contraction replicas are just three DMA reads
    of the same row at offsets 0/58/116 (REPLICATE_S2S=False) or one HBM
    read + two SBUF->SBUF shifted copies (REPLICATE_S2S=True).
  - Matmul: contraction K=96=(3 kh x 32 c) per group, two groups of a
    pair packed in the PE array via col-tiled tile_position (0,0)/(0,64),
    M=64 each; the kw shift is a column offset into the same SBUF tile.
  - kw-outer / spatial-tile-inner loop order reuses each weight load
    across 7 consecutive matmuls; the 7 spatial tiles of a wave live in
    7 PSUM banks, accumulating across the 3 kw passes.
  - PSUM->SBUF copy + bias add is split between the Vector and Scalar
    engines, writing fp16 (halves output DMA); one 831 KB DMA per wave.
"""

import numpy as np

G = 8        # groups
P = 32       # in-channels per group
F = 64       # out-channels per group
H = W = 56
HP = WP = 58           # zero-padded spatial
SP = HP * WP           # 3364 padded pixels
SHIFT = WP             # column shift of one image row
N_CORES = 8
B_PER_CORE = 2
NPAIR = G // 2
CDRAM = SP + 2 * SHIFT  # 3480: zero-bordered DRAM row
NT = 8 * SHIFT          # 464: spatial tile = 8 padded image rows
NTILE = 7               # covers padded cols [58, 3306)
NOUT = NTILE * NT       # 3248 output cols per (b, pair)

REPLICATE_S2S = False   # replicate kh on-chip instead of 3x HBM reads

_PROG_CACHE = {}


def _build_program():
    import concourse.bacc as bacc
    import concourse.mybir as mybir
    import concourse.tile as tile

    dt = mybir.dt
    nc = bacc.Bacc(
        "TRN2",
        target_bir_lowering=False,
        debug=False,
        num_devices=N_CORES,
    )

    f32 = dt.float32
    f16 = dt.float16
    IDENT = mybir.ActivationFunctionType.Identity

    # zero-bordered padded channel rows, channels-major
    xd = nc.dram_tensor("xd", [B_PER_CORE, G, P, CDRAM], f16,
                        kind="ExternalInput")
    # [p=(kh,c), g, kw, f]
    wd = nc.dram_tensor("wd", [3 * P, G, 3, F], f16, kind="ExternalInput")
    bd = nc.dram_tensor("bd", [2 * F, NPAIR], f32, kind="ExternalInput")
    outT = nc.dram_tensor("outT", [B_PER_CORE, NPAIR, 2 * F, NOUT], f16,
                          kind="ExternalOutput")

    with tile.TileContext(nc) as tc:
        with (
            tc.tile_pool(name="const", bufs=1) as cpool,
            tc.tile_pool(name="xg", bufs=2) as xpool,
            tc.tile_pool(name="ot", bufs=2) as opool,
            tc.tile_pool(name="ps", bufs=1, space="PSUM") as ppool,
        ):
            wsb = cpool.tile([3 * P, G, 3, F], f16)
            nc.sync.dma_start(wsb[:], wd[:])
            bsb = cpool.tile([2 * F, NPAIR], f32)
            nc.sync.dma_start(bsb[:], bd[:])

            def load_group(xt, b, g):
                # xt[kh*32+c, n] = padded channel row (g,c) shifted by
                # 58*(kh-1); the zero borders make the shifted reads valid.
                if not REPLICATE_S2S:
                    for kh in range(3):
                        nc.sync.dma_start(
                            xt[P * kh:P * (kh + 1), :],
                            xd[b, g, :, SHIFT * kh:SHIFT * kh + SP])
                else:
                    nc.sync.dma_start(xt[P:2 * P, :],
                                      xd[b, g, :, SHIFT:SHIFT + SP])
                    # kh=0: cols [0,58) of the shifted view are the padded
                    # bottom row (zeros), reuse them from the base's tail.
                    nc.sync.dma_start(xt[0:P, SHIFT:SP],
                                      xt[P:2 * P, 0:SP - SHIFT])
                    nc.sync.dma_start(xt[0:P, 0:SHIFT],
                                      xt[P:2 * P, SP - SHIFT:SP])
                    nc.sync.dma_start(xt[2 * P:3 * P, 0:SP - SHIFT],
                                      xt[P:2 * P, SHIFT:SP])
                    nc.sync.dma_start(xt[2 * P:3 * P, SP - SHIFT:SP],
                                      xt[P:2 * P, 0:SHIFT])

            for b in range(B_PER_CORE):
                for gp in range(NPAIR):
                    ga, gb = 2 * gp, 2 * gp + 1
                    xa = xpool.tile([3 * P, SP], f16, tag="xa")
                    xb = xpool.tile([3 * P, SP], f16, tag="xb")
                    load_group(xa, b, ga)
                    load_group(xb, b, gb)

                    osb = opool.tile([2 * F, NOUT], f16, tag="osb")
                    pss = [ppool.tile([2 * F, NT], f32, tag=f"ps{t}",
                                      name=f"ps{t}")
                           for t in range(NTILE)]
                    for dw in range(3):
                        # all 7 tiles of a col-group back-to-back so the
                        # stationary weights are reloaded only on change
                        for t in range(NTILE):
                            s = (1 + 8 * t) * SHIFT
                            nc.tensor.matmul(
                                pss[t][0:F, :],
                                wsb[:, ga, dw, :],
                                xa[:, s - 1 + dw:s - 1 + dw + NT],
                                start=(dw == 0), stop=(dw == 2),
                                tile_position=(0, 0),
                            )
                        for t in range(NTILE):
                            s = (1 + 8 * t) * SHIFT
                            nc.tensor.matmul(
                                pss[t][F:2 * F, :],
                                wsb[:, gb, dw, :],
                                xb[:, s - 1 + dw:s - 1 + dw + NT],
                                start=(dw == 0), stop=(dw == 2),
                                tile_position=(0, F),
                            )
                    for t in range(NTILE):
                        dst = osb[:, t * NT:(t + 1) * NT]
                        if t % 2 == 0:
                            nc.vector.tensor_scalar_add(
                                dst, pss[t][:, :], bsb[:, gp:gp + 1])
                        else:
                            nc.scalar.activation(
                                dst, pss[t][:, :], IDENT,
                                bias=bsb[:, gp:gp + 1], scale=1.0)
                    nc.sync.dma_start(outT[b, gp, :, :], osb[:])

    nc.compile()
    return nc


def _get_program():
    if "nc" not in _PROG_CACHE:
        _PROG_CACHE["nc"] = _build_program()
    return _PROG_CACHE["nc"]


def prepare_in_maps(x, kernels, bias):
    x = np.ascontiguousarray(x, dtype=np.float32)
    kernels = np.ascontiguousarray(kernels, dtype=np.float32)
    bias = np.ascontiguousarray(bias, dtype=np.float32)
    nb = x.shape[0]

    # zero-bordered padded channel rows: [nb, g, c, 3480]
    xc = x.transpose(0, 3, 1, 2)                       # [nb, 256, 56, 56]
    xpad = np.zeros((nb, G * P, HP, WP), np.float16)
    xpad[:, :, 1:1 + H, 1:1 + W] = xc.astype(np.float16)
    xd = np.zeros((nb, G, P, CDRAM), np.float16)
    xd[:, :, :, SHIFT:SHIFT + SP] = xpad.reshape(nb, G, P, SP)

    # [p=(kh,c), g, kw, f]
    wd = np.ascontiguousarray(
        kernels.transpose(1, 3, 0, 2, 4).reshape(3 * P, G, 3, F)
    ).astype(np.float16)

    bd = np.ascontiguousarray(bias.reshape(NPAIR, 2 * F).T)

    return [
        {"xd": np.ascontiguousarray(xd[i * B_PER_CORE:(i + 1) * B_PER_CORE]),
         "wd": wd, "bd": bd}
        for i in range(N_CORES)
    ]


def gather_output(results, nb):
    out = np.empty((nb, H, W, G * F), np.float32)
    for i in range(N_CORES):
        o = results[i]["outT"].astype(np.float32)      # [2, 4, 128, 3248]
        o = o.reshape(B_PER_CORE, NPAIR, 2 * F, H, WP)[:, :, :, :, 1:1 + W]
        out[i * B_PER_CORE:(i + 1) * B_PER_CORE] = (
            o.transpose(0, 3, 4, 1, 2).reshape(B_PER_CORE, H, W, G * F))
    return out


def kernel(x, kernels, bias):
    from concourse.bass_utils import run_bass_kernel_spmd

    nc = _get_program()
    in_maps = prepare_in_maps(x, kernels, bias)
    res = run_bass_kernel_spmd(nc, in_maps, list(range(N_CORES)))
    return gather_output(res.results, np.asarray(x).shape[0])


# revision 5
# speedup vs baseline: 1.0160x; 1.0160x over previous
"""Grouped 3x3 SAME conv on 8 Trainium2 NeuronCores.

Problem: x[16,56,56,256] NHWC, 8 groups of 32->64 channels, 3x3 SAME,
out[16,56,56,512], fp32.

Strategy (hardcoded):
  - Data-parallel over batch: core i handles images [2i, 2i+1].
  - fp16 operands (fp32 PSUM accumulate, rel err ~3e-4). Input DRAM rows
    are zero-bordered padded channel rows [58 zeros | img 3364 | 58 zeros]
    so the three kh-shifted contraction replicas are just three DMA reads
    of the same row at offsets 0/58/116 (REPLICATE_S2S=False) or one HBM
    read + two SBUF->SBUF shifted copies (REPLICATE_S2S=True).
  - Matmul: contraction K=96=(3 kh x 32 c) per group, two groups of a
    pair packed in the PE array via col-tiled tile_position (0,0)/(0,64),
    M=64 each; the kw shift is a column offset into the same SBUF tile.
  - kw-outer / spatial-tile-inner loop order reuses each weight load
    across 7 consecutive matmuls; the 7 spatial tiles of a wave live in
    7 PSUM banks, accumulating across the 3 kw passes.
  - PSUM->SBUF copy + bias add is split between the Vector and Scalar
    engines, writing fp16 (halves output DMA); one 831 KB DMA per wave.
"""

import numpy as np

G = 8        # groups
P = 32       # in-channels per group
F = 64       # out-channels per group
H = W = 56
HP = WP = 58           # zero-padded spatial
SP = HP * WP           # 3364 padded pixels
SHIFT = WP             # column shift of one image row
N_CORES = 8
B_PER_CORE = 2
NPAIR = G // 2
CDRAM = SP + 2 * SHIFT  # 3480: zero-bordered DRAM row
NT = 8 * SHIFT          # 464: spatial tile = 8 padded image rows
NTILE = 7               # covers padded cols [58, 3306)
NOUT = NTILE * NT       # 3248 output cols per (b, pair)

REPLICATE_S2S = False   # replicate kh on-chip instead of 3x HBM reads

_PROG_CACHE = {}


def _build_program():
    import concourse.bacc as bacc
    import concourse.mybir as mybir
    import concourse.tile as tile

    dt = mybir.dt
    nc = bacc.Bacc(
        "TRN2",
        target_bir_lowering=False,
        debug=False,
        num_devices=N_CORES,
    )

    f32 = dt.float32
    f16 = dt.float16
    IDENT = mybir.ActivationFunctionType.Identity

    # zero-bordered padded channel rows, channels-major
    xd = nc.dram_tensor("xd", [B_PER_CORE, G, P, CDRAM], f16,
                        kind="ExternalInput")
    # [p=(kh,c), g, kw, f]
    wd = nc.dram_tensor("wd", [3 * P, G, 3, F], f16, kind="ExternalInput")
    bd = nc.dram_tensor("bd", [2 * F, NPAIR], f32, kind="ExternalInput")
    outT = nc.dram_tensor("outT", [B_PER_CORE, NPAIR, 2 * F, NOUT], f16,
                          kind="ExternalOutput")

    with tile.TileContext(nc) as tc:
        with (
            tc.tile_pool(name="const", bufs=1) as cpool,
            tc.tile_pool(name="xg", bufs=2) as xpool,
            tc.tile_pool(name="ot", bufs=2) as opool,
            tc.tile_pool(name="ps", bufs=1, space="PSUM") as ppool,
        ):
            wsb = cpool.tile([3 * P, G, 3, F], f16)
            nc.sync.dma_start(wsb[:], wd[:])
            bsb = cpool.tile([2 * F, NPAIR], f32)
            nc.sync.dma_start(bsb[:], bd[:])

            def load_group(xt, b, g):
                # xt[kh*32+c, n] = padded channel row (g,c) shifted by
                # 58*(kh-1); the zero borders make the shifted reads valid.
                if not REPLICATE_S2S:
                    for kh in range(3):
                        nc.sync.dma_start(
                            xt[P * kh:P * (kh + 1), :],
                            xd[b, g, :, SHIFT * kh:SHIFT * kh + SP])
                else:
                    nc.sync.dma_start(xt[P:2 * P, :],
                                      xd[b, g, :, SHIFT:SHIFT + SP])
                    # kh=0: cols [0,58) of the shifted view are the padded
                    # bottom row (zeros), reuse them from the base's tail.
                    nc.sync.dma_start(xt[0:P, SHIFT:SP],
                                      xt[P:2 * P, 0:SP - SHIFT])
                    nc.sync.dma_start(xt[0:P, 0:SHIFT],
                                      xt[P:2 * P, SP - SHIFT:SP])
                    nc.sync.dma_start(xt[2 * P:3 * P, 0:SP - SHIFT],
                                      xt[P:2 * P, SHIFT:SP])
                    nc.sync.dma_start(xt[2 * P:3 * P, SP - SHIFT:SP],
                                      xt[P:2 * P, 0:SHIFT])

            for b in range(B_PER_CORE):
                for gp in range(NPAIR):
                    ga, gb = 2 * gp, 2 * gp + 1
                    xa = xpool.tile([3 * P, SP], f16, tag="xa")
                    xb = xpool.tile([3 * P, SP], f16, tag="xb")
                    load_group(xa, b, ga)
                    load_group(xb, b, gb)

                    osb = opool.tile([2 * F, NOUT], f16, tag="osb")
                    pss = [ppool.tile([2 * F, NT], f32, tag=f"ps{t}",
                                      name=f"ps{t}")
                           for t in range(NTILE)]
                    for dw in range(3):
                        # a/b interleaved per tile: matmul starts are
                        # pc-monotone, so the (0,0)/(0,64) col-tiled pair
                        # only overlaps when issued back-to-back
                        for t in range(NTILE):
                            s = (1 + 8 * t) * SHIFT
                            nc.tensor.matmul(
                                pss[t][0:F, :],
                                wsb[:, ga, dw, :],
                                xa[:, s - 1 + dw:s - 1 + dw + NT],
                                start=(dw == 0), stop=(dw == 2),
                                tile_position=(0, 0),
                            )
                            nc.tensor.matmul(
                                pss[t][F:2 * F, :],
                                wsb[:, gb, dw, :],
                                xb[:, s - 1 + dw:s - 1 + dw + NT],
                                start=(dw == 0), stop=(dw == 2),
                                tile_position=(0, F),
                            )
                    for t in range(NTILE):
                        dst = osb[:, t * NT:(t + 1) * NT]
                        if t % 2 == 0:
                            nc.vector.tensor_scalar_add(
                                dst, pss[t][:, :], bsb[:, gp:gp + 1])
                        else:
                            nc.scalar.activation(
                                dst, pss[t][:, :], IDENT,
                                bias=bsb[:, gp:gp + 1], scale=1.0)
                    nc.sync.dma_start(outT[b, gp, :, :], osb[:])

    nc.compile()
    return nc


def _get_program():
    if "nc" not in _PROG_CACHE:
        _PROG_CACHE["nc"] = _build_program()
    return _PROG_CACHE["nc"]


def prepare_in_maps(x, kernels, bias):
    x = np.ascontiguousarray(x, dtype=np.float32)
    kernels = np.ascontiguousarray(kernels, dtype=np.float32)
    bias = np.ascontiguousarray(bias, dtype=np.float32)
    nb = x.shape[0]

    # zero-bordered padded channel rows: [nb, g, c, 3480]
    xc = x.transpose(0, 3, 1, 2)                       # [nb, 256, 56, 56]
    xpad = np.zeros((nb, G * P, HP, WP), np.float16)
    xpad[:, :, 1:1 + H, 1:1 + W] = xc.astype(np.float16)
    xd = np.zeros((nb, G, P, CDRAM), np.float16)
    xd[:, :, :, SHIFT:SHIFT + SP] = xpad.reshape(nb, G, P, SP)

    # [p=(kh,c), g, kw, f]
    wd = np.ascontiguousarray(
        kernels.transpose(1, 3, 0, 2, 4).reshape(3 * P, G, 3, F)
    ).astype(np.float16)

    bd = np.ascontiguousarray(bias.reshape(NPAIR, 2 * F).T)

    return [
        {"xd": np.ascontiguousarray(xd[i * B_PER_CORE:(i + 1) * B_PER_CORE]),
         "wd": wd, "bd": bd}
        for i in range(N_CORES)
    ]


def gather_output(results, nb):
    out = np.empty((nb, H, W, G * F), np.float32)
    for i in range(N_CORES):
        o = results[i]["outT"].astype(np.float32)      # [2, 4, 128, 3248]
        o = o.reshape(B_PER_CORE, NPAIR, 2 * F, H, WP)[:, :, :, :, 1:1 + W]
        out[i * B_PER_CORE:(i + 1) * B_PER_CORE] = (
            o.transpose(0, 3, 4, 1, 2).reshape(B_PER_CORE, H, W, G * F))
    return out


def kernel(x, kernels, bias):
    from concourse.bass_utils import run_bass_kernel_spmd

    nc = _get_program()
    in_maps = prepare_in_maps(x, kernels, bias)
    res = run_bass_kernel_spmd(nc, in_maps, list(range(N_CORES)))
    return gather_output(res.results, np.asarray(x).shape[0])


# revision 6
# speedup vs baseline: 1.5201x; 1.4962x over previous
"""Grouped 3x3 SAME conv on 8 Trainium2 NeuronCores.

Problem: x[16,56,56,256] NHWC, 8 groups of 32->64 channels, 3x3 SAME,
out[16,56,56,512], fp32.

Strategy (hardcoded):
  - Data-parallel over batch: core i handles images [2i, 2i+1].
  - fp16 operands (fp32 PSUM accumulate, rel err ~3e-4). Each DRAM input
    row holds BOTH images' zero-bordered padded channel rows
    back-to-back ([58z|img0|58z|58z|img1|58z] = 6960 cols), so one DMA
    read at column offset 58*kh yields a 13.7 KB contiguous line per
    partition covering both images' kh-shifted views (large packets keep
    the 16 SDMA engines near line rate). Three such reads at offsets
    0/58/116 build the kh-replicated contraction tile.
  - Input DMAs for pair gp+1 are issued (on the Sync queue) before pair
    gp's compute so the loads overlap compute and the PE never idles past
    the HAM re-throttle window; the output DMA rides the otherwise-idle
    GpSimd SWDGE queue so its semaphore wait can't block the prefetch.
  - Matmul: contraction K=96=(3 kh x 32 c) per group, groups of a pair
    at PE col-halves via tile_position (0,0)/(0,64); the kw shift is a
    column offset into the same SBUF tile. kw-outer / tile-inner order;
    7 spatial tiles per image live in 7 PSUM banks.
  - PSUM->SBUF copy + bias add alternates Vector/Scalar engines, writes
    fp16; one 1.66 MB output DMA per pair.
"""

import numpy as np

G = 8        # groups
P = 32       # in-channels per group
F = 64       # out-channels per group
H = W = 56
HP = WP = 58           # zero-padded spatial
SP = HP * WP           # 3364 padded pixels
SHIFT = WP             # column shift of one image row
N_CORES = 8
B_PER_CORE = 2
NPAIR = G // 2
CROW = SP + 2 * SHIFT   # 3480: one image's zero-bordered row
CDRAM = B_PER_CORE * CROW       # 6960: both images back-to-back
CVIEW = CROW + SP               # 6844: shifted window over both images
NT = 8 * SHIFT          # 464: spatial tile = 8 padded image rows
NTILE = 7               # covers padded cols [58, 3306)
NOUT = NTILE * NT       # 3248 output cols per image

_PROG_CACHE = {}


def _build_program():
    import concourse.bacc as bacc
    import concourse.mybir as mybir
    import concourse.tile as tile

    dt = mybir.dt
    nc = bacc.Bacc(
        "TRN2",
        target_bir_lowering=False,
        debug=False,
        num_devices=N_CORES,
    )

    f32 = dt.float32
    f16 = dt.float16
    IDENT = mybir.ActivationFunctionType.Identity

    xd = nc.dram_tensor("xd", [G, P, CDRAM], f16, kind="ExternalInput")
    # [p=(kh,c), g, kw, f]
    wd = nc.dram_tensor("wd", [3 * P, G, 3, F], f16, kind="ExternalInput")
    bd = nc.dram_tensor("bd", [2 * F, NPAIR], f32, kind="ExternalInput")
    outT = nc.dram_tensor("outT", [NPAIR, 2 * F, B_PER_CORE, NOUT], f16,
                          kind="ExternalOutput")

    with tile.TileContext(nc) as tc:
        with (
            tc.tile_pool(name="const", bufs=1) as cpool,
            tc.tile_pool(name="xg", bufs=2) as xpool,
            tc.tile_pool(name="ot", bufs=2) as opool,
            tc.tile_pool(name="ps", bufs=1, space="PSUM") as ppool,
        ):
            wsb = cpool.tile([3 * P, G, 3, F], f16)
            nc.sync.dma_start(wsb[:], wd[:])
            bsb = cpool.tile([2 * F, NPAIR], f32)
            nc.sync.dma_start(bsb[:], bd[:])

            def load_pair(gp):
                xa = xpool.tile([3 * P, CVIEW], f16, tag="xa", name="xa")
                xb = xpool.tile([3 * P, CVIEW], f16, tag="xb", name="xb")
                for xt, g in ((xa, 2 * gp), (xb, 2 * gp + 1)):
                    for kh in range(3):
                        nc.sync.dma_start(
                            xt[P * kh:P * (kh + 1), :],
                            xd[g, :, SHIFT * kh:SHIFT * kh + CVIEW])
                return xa, xb

            nxt = load_pair(0)
            for gp in range(NPAIR):
                xa, xb = nxt
                if gp + 1 < NPAIR:
                    nxt = load_pair(gp + 1)

                osb = opool.tile([2 * F, B_PER_CORE * NOUT], f16, tag="osb")
                for img in range(B_PER_CORE):
                    base = CROW * img
                    pss = [ppool.tile([2 * F, NT], f32, tag=f"ps{t}",
                                      name=f"ps{t}")
                           for t in range(NTILE)]
                    for dw in range(3):
                        for t in range(NTILE):
                            o = base + (1 + 8 * t) * SHIFT - 1 + dw
                            nc.tensor.matmul(
                                pss[t][0:F, :],
                                wsb[:, 2 * gp, dw, :],
                                xa[:, o:o + NT],
                                start=(dw == 0), stop=(dw == 2),
                                tile_position=(0, 0),
                            )
                            nc.tensor.matmul(
                                pss[t][F:2 * F, :],
                                wsb[:, 2 * gp + 1, dw, :],
                                xb[:, o:o + NT],
                                start=(dw == 0), stop=(dw == 2),
                                tile_position=(0, F),
                            )
                    for t in range(NTILE):
                        dst = osb[:, img * NOUT + t * NT:
                                  img * NOUT + (t + 1) * NT]
                        if t % 2 == 0:
                            nc.vector.tensor_scalar_add(
                                dst, pss[t][:, :], bsb[:, gp:gp + 1])
                        else:
                            nc.scalar.activation(
                                dst, pss[t][:, :], IDENT,
                                bias=bsb[:, gp:gp + 1], scale=1.0)
                nc.gpsimd.dma_start(outT[gp, :, :, :], osb[:])

    nc.compile()
    return nc


def _get_program():
    if "nc" not in _PROG_CACHE:
        _PROG_CACHE["nc"] = _build_program()
    return _PROG_CACHE["nc"]


def prepare_in_maps(x, kernels, bias):
    x = np.ascontiguousarray(x, dtype=np.float32)
    kernels = np.ascontiguousarray(kernels, dtype=np.float32)
    bias = np.ascontiguousarray(bias, dtype=np.float32)
    nb = x.shape[0]

    # zero-bordered padded channel rows, both images of a core
    # back-to-back: [nb//2, g, c, 2*3480]
    xc = x.transpose(0, 3, 1, 2)                       # [nb, 256, 56, 56]
    xpad = np.zeros((nb, G * P, HP, WP), np.float16)
    xpad[:, :, 1:1 + H, 1:1 + W] = xc.astype(np.float16)
    xrow = np.zeros((nb, G, P, CROW), np.float16)
    xrow[:, :, :, SHIFT:SHIFT + SP] = xpad.reshape(nb, G, P, SP)
    xd = (xrow.reshape(N_CORES, B_PER_CORE, G, P, CROW)
              .transpose(0, 2, 3, 1, 4)
              .reshape(N_CORES, G, P, CDRAM))

    # [p=(kh,c), g, kw, f]
    wd = np.ascontiguousarray(
        kernels.transpose(1, 3, 0, 2, 4).reshape(3 * P, G, 3, F)
    ).astype(np.float16)

    bd = np.ascontiguousarray(bias.reshape(NPAIR, 2 * F).T)

    return [
        {"xd": np.ascontiguousarray(xd[i]), "wd": wd, "bd": bd}
        for i in range(N_CORES)
    ]


def gather_output(results, nb):
    out = np.empty((nb, H, W, G * F), np.float32)
    for i in range(N_CORES):
        o = results[i]["outT"].astype(np.float32)  # [4, 128, 2, 3248]
        o = o.reshape(NPAIR, 2 * F, B_PER_CORE, H, WP)[:, :, :, :, 1:1 + W]
        out[i * B_PER_CORE:(i + 1) * B_PER_CORE] = (
            o.transpose(2, 3, 4, 0, 1).reshape(B_PER_CORE, H, W, G * F))
    return out


def kernel(x, kernels, bias):
    from concourse.bass_utils import run_bass_kernel_spmd

    nc = _get_program()
    in_maps = prepare_in_maps(x, kernels, bias)
    res = run_bass_kernel_spmd(nc, in_maps, list(range(N_CORES)))
    return gather_output(res.results, np.asarray(x).shape[0])
